# revision 1
# baseline (speedup 1.0000x reference)
"""Trainium2 Bass kernel for nn_CrossAttentionFusion (cross-attention + BitLinear FFN).

Sharding: 8 cores = 4 batches x 2 sequence-halves. Each core:
  - owns 1024 query tokens (sem shard, feature-major),
  - computes K/V for its batch's full 2048 tokens from pro (feature-major),
  - runs full attention for its queries + BitLinear FFN, writes its out^T shard.
No collectives needed; host does all layout transposes and the final gather.
"""
import math
import numpy as np
from contextlib import ExitStack

import concourse.bass as bass
import concourse.bass_isa as bass_isa
import concourse.tile as tile
from concourse import bacc, mybir
from concourse.bass_utils import run_bass_kernel_spmd

F32 = mybir.dt.float32
BF16 = mybir.dt.bfloat16
FP8 = mybir.dt.float8e4
AF = mybir.ActivationFunctionType
ALU = mybir.AluOpType

B, S, DS, DP, H = 4, 2048, 1024, 512, 8
DF = 4 * DS
HD = DS // H          # 128
TOK = 1024            # query tokens per core
N_CORES = 8
EPS = 1e-6
C_RND = 12582912.0    # 1.5 * 2**23 : +C-C = round-to-nearest-even
QK_SCALE = 1.0 / math.sqrt(HD)

P = 128
M_SEM = DS // P       # 8
M_PRO = DP // P       # 4
M_FF = DF // P        # 32
NT_Q = TOK // 512     # 2
NT_K = S // P         # 16
MT_V = S // P         # 16


def bcast_free(ap2d, rep):
    """[P, W] AP -> [P, rep, W] AP with step-0 middle dim (free broadcast)."""
    return bass.AP(tensor=ap2d.tensor, offset=ap2d.offset,
                   ap=[ap2d.ap[0], [0, rep], ap2d.ap[1]])


def build_nc(debug_outs=False):
    nc = bacc.Bacc("TRN2", target_bir_lowering=False, debug=False, num_devices=N_CORES)

    semT = nc.dram_tensor("semT", [DS, TOK], F32, kind="ExternalInput").ap()
    proT = nc.dram_tensor("proT", [DP, S], F32, kind="ExternalInput").ap()
    wqT = nc.dram_tensor("wqT", [DS, DS], BF16, kind="ExternalInput").ap()
    wkT = nc.dram_tensor("wkT", [DP, DS], BF16, kind="ExternalInput").ap()
    wvT = nc.dram_tensor("wvT", [DP, DS], BF16, kind="ExternalInput").ap()
    woT = nc.dram_tensor("woT", [DS, DS], BF16, kind="ExternalInput").ap()
    w1T = nc.dram_tensor("w1T", [DS, DF], F32, kind="ExternalInput").ap()
    w2T = nc.dram_tensor("w2T", [DF, DS], F32, kind="ExternalInput").ap()
    w1s = nc.dram_tensor("w1s", [P, DF], F32, kind="ExternalInput").ap()
    w2s = nc.dram_tensor("w2s", [DP, DS], F32, kind="ExternalInput").ap()
    gsem = nc.dram_tensor("gsem", [P, M_SEM], F32, kind="ExternalInput").ap()
    gpro = nc.dram_tensor("gpro", [P, M_PRO], F32, kind="ExternalInput").ap()
    gff = nc.dram_tensor("gff", [P, M_SEM], F32, kind="ExternalInput").ap()
    bq = nc.dram_tensor("bq", [P, M_SEM], F32, kind="ExternalInput").ap()
    bk = nc.dram_tensor("bk", [P, M_SEM], F32, kind="ExternalInput").ap()
    bv = nc.dram_tensor("bv", [P, M_SEM], F32, kind="ExternalInput").ap()
    bo = nc.dram_tensor("bo", [P, M_SEM], F32, kind="ExternalInput").ap()
    alpha = nc.dram_tensor("alpha", [P, M_FF], F32, kind="ExternalInput").ap()
    beta = nc.dram_tensor("beta", [P, M_FF], F32, kind="ExternalInput").ap()
    outT = nc.dram_tensor("outT", [DS, TOK], F32, kind="ExternalOutput").ap()

    dbg = {}
    if debug_outs:
        for name, shape, dt in [
            ("dbg_semn", [DS, TOK], BF16), ("dbg_q", [DS, TOK], BF16),
            ("dbg_k", [DS, S], BF16), ("dbg_v", [S, DS], BF16),
            ("dbg_ctx", [DS, TOK], BF16), ("dbg_semout", [DS, TOK], F32),
            ("dbg_xq", [DS, TOK], BF16), ("dbg_h", [DF, TOK], BF16),
            ("dbg_hq", [DF, TOK], BF16), ("dbg_mw", [1, 2], F32),
        ]:
            dbg[name] = nc.dram_tensor(name, shape, dt, kind="ExternalOutput").ap()

    with tile.TileContext(nc) as tc, ExitStack() as top:
        persist = top.enter_context(tc.tile_pool(name="persist", bufs=1))
        rows = top.enter_context(tc.tile_pool(name="rows", bufs=1))
        ps_mm = top.enter_context(tc.tile_pool(name="ps_mm", bufs=2, space="PSUM"))
        pdram_w = top.enter_context(tc.tile_pool(name="pdram_w", bufs=1,
                                                 space="DRAM"))
        w1q_d = pdram_w.tile([P, M_FF, M_SEM, P], BF16)
        w2q_d = pdram_w.tile([P, M_SEM, M_FF, P], BF16)

        ones = persist.tile([P, 1], BF16)
        nc.vector.memset(ones[:], 1.0)
        ones_row = persist.tile([1, P], BF16)
        nc.vector.memset(ones_row[:], 1.0)
        eps_t = persist.tile([1, 1], F32)
        nc.vector.memset(eps_t[:], EPS)

        gsem_sb = persist.tile([P, M_SEM], F32)
        gpro_sb = persist.tile([P, M_PRO], F32)
        gff_sb = persist.tile([P, M_SEM], F32)
        bq_sb = persist.tile([P, M_SEM], F32)
        bk_sb = persist.tile([P, M_SEM], F32)
        bv_sb = persist.tile([P, M_SEM], F32)
        bo_sb = persist.tile([P, M_SEM], F32)
        alpha_sb = persist.tile([P, M_FF], F32)
        rbeta_sb = persist.tile([P, M_FF], F32)
        for ap_d, t in [(gsem, gsem_sb), (gpro, gpro_sb), (gff, gff_sb),
                        (bq, bq_sb), (bk, bk_sb), (bv, bv_sb), (bo, bo_sb),
                        (alpha, alpha_sb)]:
            nc.sync.dma_start(t[:], ap_d[:])
        beta_t = persist.tile([P, M_FF], F32)
        nc.sync.dma_start(beta_t[:], beta[:])
        nc.vector.tensor_scalar(rbeta_sb[:], beta_t[:], 1e-9, None, ALU.add)
        nc.vector.reciprocal(rbeta_sb[:], rbeta_sb[:])

        semT_r = semT.rearrange("(m p) t -> p m t", p=P)

        def rmsnorm_fm(pool, fetch, nm, T, g_sb, out_bf):
            """feature-major rmsnorm: out_bf[:, m, :] = x_m * g_m * rsqrt(ms+eps)"""
            D = nm * P
            rs_row = pool.tile([1, T], F32, tag="rs_row", bufs=1)
            xs = [fetch(m) for m in range(nm)]
            for ch in range(T // 512):
                pst = ps_mm.tile([P, 512], F32, tag="mm")
                ps = pst[0:1, :]
                for m in range(nm):
                    sq = pool.tile([P, 512], BF16, tag="sq", bufs=3)
                    nc.scalar.activation(sq[:], xs[m][:, ch * 512:(ch + 1) * 512],
                                         AF.Square)
                    nc.tensor.matmul(ps[:], ones[:], sq[:],
                                     start=(m == 0), stop=(m == nm - 1))
                nc.scalar.activation(rs_row[:, ch * 512:(ch + 1) * 512], ps[:],
                                     AF.Ln, bias=eps_t[:], scale=1.0 / D)
            nc.scalar.activation(rs_row[:], rs_row[:], AF.Exp, scale=-0.5)
            rs_bc = pool.tile([P, T], F32, tag="rs_bc", bufs=1)
            nc.gpsimd.partition_broadcast(rs_bc[:], rs_row[:])
            for m in range(nm):
                nc.vector.scalar_tensor_tensor(
                    out=out_bf[:, m, :], in0=xs[m][:],
                    scalar=g_sb[:, m:m + 1], in1=rs_bc[:],
                    op0=ALU.mult, op1=ALU.mult)

        # ================= phase 1: input norms =================
        es_norm = ExitStack()
        pnorm = es_norm.enter_context(tc.tile_pool(name="pnorm", bufs=1))
        semn_sb = pnorm.tile([P, M_SEM, TOK], BF16)
        pron_sb = pnorm.tile([P, M_PRO, S], BF16)

        with tc.tile_pool(name="pin1", bufs=1) as pin1:
            semT_sb = pin1.tile([P, M_SEM, TOK], F32)
            nc.sync.dma_start(semT_sb[:], semT_r)
            rmsnorm_fm(pin1, lambda m: semT_sb[:, m, :], M_SEM, TOK, gsem_sb, semn_sb)

        with tc.tile_pool(name="pin2", bufs=1, side="right") as pin2:
            proT_sb = pin2.tile([P, M_PRO, S], F32)
            nc.sync.dma_start(proT_sb[:], proT.rearrange("(m p) t -> p m t", p=P))
            rmsnorm_fm(pin2, lambda m: proT_sb[:, m, :], M_PRO, S, gpro_sb, pron_sb)

        if debug_outs:
            nc.sync.dma_start(dbg["dbg_semn"].rearrange("(m p) t -> p m t", p=P),
                              semn_sb[:])

        # ================= phase 3: Q/K/V =================
        es_qkv = ExitStack()
        pqkv = es_qkv.enter_context(tc.tile_pool(name="pqkv", bufs=1, side="right"))
        q_sb = pqkv.tile([P, M_SEM, TOK], FP8)
        k_sb = pqkv.tile([P, M_SEM, S], FP8)
        v_sb = pqkv.tile([P, MT_V, DS], BF16)

        with tc.tile_pool(name="pw3", bufs=1) as pw3:
            wq_sb = pw3.tile([P, M_SEM, DS], BF16)
            nc.sync.dma_start(wq_sb[:], wqT.rearrange("(m p) o -> p m o", p=P))
            for m in range(M_SEM):
                for n in range(NT_Q):
                    ps = ps_mm.tile([P, 512], F32, tag="mm")
                    for kk in range(M_SEM):
                        nc.tensor.matmul(ps[:], wq_sb[:, kk, m * P:(m + 1) * P],
                                         semn_sb[:, kk, n * 512:(n + 1) * 512],
                                         start=(kk == 0), stop=(kk == M_SEM - 1))
                    nc.scalar.activation(q_sb[:, m, n * 512:(n + 1) * 512], ps[:],
                                         AF.Identity, bias=bq_sb[:, m:m + 1])

            wk_sb = pw3.tile([P, M_PRO, DS], BF16)
            nc.sync.dma_start(wk_sb[:], wkT.rearrange("(m p) o -> p m o", p=P))
            for m in range(M_SEM):
                for n in range(S // 512):
                    ps = ps_mm.tile([P, 512], F32, tag="mm")
                    for kk in range(M_PRO):
                        nc.tensor.matmul(ps[:], wk_sb[:, kk, m * P:(m + 1) * P],
                                         pron_sb[:, kk, n * 512:(n + 1) * 512],
                                         start=(kk == 0), stop=(kk == M_PRO - 1))
                    nc.scalar.activation(k_sb[:, m, n * 512:(n + 1) * 512], ps[:],
                                         AF.Identity, bias=bk_sb[:, m:m + 1])

            wv_sb = pw3.tile([P, M_PRO, DS], BF16)
            nc.sync.dma_start(wv_sb[:], wvT.rearrange("(m p) o -> p m o", p=P))
            for mt in range(MT_V):
                for n in range(DS // 512):
                    ps = ps_mm.tile([P, 512], F32, tag="mm")
                    for kk in range(M_PRO):
                        nc.tensor.matmul(ps[:], pron_sb[:, kk, mt * P:(mt + 1) * P],
                                         wv_sb[:, kk, n * 512:(n + 1) * 512],
                                         start=(kk == 0), stop=(kk == M_PRO - 1))
                    # bias bv folded in at ctx evac
                    nc.scalar.activation(v_sb[:, mt, n * 512:(n + 1) * 512], ps[:],
                                         AF.Copy)
        es_norm.close()   # semn/pron freed

        # ===== phase 2: mean(|w|) via per-core strips + AllReduce =====
        with tc.tile_pool(name="pwmean", bufs=2) as pwm, \
             tc.tile_pool(name="pdram", bufs=1, space="DRAM") as pdram:
            def strip_sum(ws_ap, nrows, cols, name):
                ntile = nrows // P
                nch = cols // 1024
                mcols = rows.tile([P, ntile * nch], F32, tag=f"mcols_{name}")
                for j in range(ntile):
                    for ci in range(nch):
                        wt = pwm.tile([P, 1024], F32, tag="wmean")
                        nc.sync.dma_start(
                            wt[:], ws_ap[j * P:(j + 1) * P,
                                         ci * 1024:(ci + 1) * 1024])
                        nc.scalar.activation(wt[:], wt[:], AF.Abs,
                                             accum_out=mcols[:, j * nch + ci:
                                                             j * nch + ci + 1])
                msum = rows.tile([P, 1], F32, tag=f"msum_{name}")
                nc.vector.tensor_reduce(msum[:], mcols[:], axis=mybir.AxisListType.X,
                                        op=ALU.add)
                msum_all = rows.tile([P, 1], F32, tag=f"msuma_{name}")
                nc.gpsimd.partition_all_reduce(msum_all[:], msum[:], P,
                                               bass_isa.ReduceOp.add)
                return msum_all

            s1 = strip_sum(w1s, P, DF, "w1")
            s2 = strip_sum(w2s, DP, DS, "w2")
            loc = rows.tile([1, 2], F32, tag="ccloc")
            nc.vector.tensor_copy(loc[:, 0:1], s1[0:1, :])
            nc.vector.tensor_copy(loc[:, 1:2], s2[0:1, :])
            cin = pdram.tile([1, 2], F32)
            cout = pdram.tile([1, 2], F32)
            nc.sync.dma_start(cin[:], loc[:])
            nc.gpsimd.collective_compute(
                "AllReduce", ALU.add,
                replica_groups=[list(range(N_CORES))],
                ins=[cin.opt()], outs=[cout.opt()])
            tot = rows.tile([1, 2], F32, tag="cctot")
            nc.sync.dma_start(tot[:], cout[:])
            mwrow = rows.tile([1, 2], F32, tag="mwrow")
            nc.vector.tensor_scalar(mwrow[:, 0:1], tot[:, 0:1], 1.0 / (DS * DF),
                                    None, ALU.mult)
            nc.vector.tensor_scalar(mwrow[:, 1:2], tot[:, 1:2], 1.0 / (DF * DS),
                                    None, ALU.mult)
            mw_all = rows.tile([P, 2], F32, tag="mwall")
            nc.gpsimd.partition_broadcast(mw_all[:], mwrow[:])
            mw1, mw2 = mw_all[:, 0:1], mw_all[:, 1:2]
            sw_all = rows.tile([P, 2], F32, tag="swall")
            nc.vector.reciprocal(sw_all[:], mw_all[:])
            sw1_bc, sw2_bc = sw_all[:, 0:1], sw_all[:, 1:2]
        if debug_outs:
            nc.sync.dma_start(dbg["dbg_mw"][:], mwrow[:])

        # folded snake scalars: alphap = alpha*mw1 ; rbetap = rbeta/mw1
        alphap = persist.tile([P, M_FF], F32)
        rbetap = persist.tile([P, M_FF], F32)
        nc.vector.tensor_scalar(alphap[:], alpha_sb[:], mw1, None, ALU.mult)
        nc.vector.tensor_scalar(rbetap[:], rbeta_sb[:], sw1_bc, None, ALU.mult)

        # pre-ternarize W1/W2 into DRAM (overlaps QKV/attention)
        with tc.tile_pool(name="ptern", bufs=1, side="right") as ptern:
            w1r_ = w1T.rearrange("(kt p) o -> p kt o", p=P)
            for m in range(M_FF):
                wc = ptern.tile([P, M_SEM, P], F32, tag="w1c", bufs=1)
                nc.sync.dma_start(wc[:], w1r_[:, :, m * P:(m + 1) * P])
                tw = ptern.tile([P, M_SEM * P], F32, tag="terntmp", bufs=1)
                wcf = wc[:].rearrange("p a b -> p (a b)")
                nc.vector.tensor_scalar(tw[:], wcf, sw1_bc, None, ALU.mult)
                nc.vector.tensor_scalar(tw[:], tw[:], 1.49, -1.49, ALU.min,
                                        ALU.max)
                w1q = ptern.tile([P, M_SEM, P], BF16, tag="w1q", bufs=1)
                nc.vector.tensor_scalar(w1q[:].rearrange("p a b -> p (a b)"),
                                        tw[:], C_RND, C_RND, ALU.add,
                                        ALU.subtract)
                nc.sync.dma_start(w1q_d[:, m], w1q[:])
            w2r_ = w2T.rearrange("(kt p) o -> p kt o", p=P)
            for m in range(M_SEM):
                for sub in range(4):
                    wc2 = ptern.tile([P, M_SEM, P], F32, tag="w1c", bufs=1)
                    nc.sync.dma_start(
                        wc2[:], w2r_[:, sub * M_SEM:(sub + 1) * M_SEM,
                                     m * P:(m + 1) * P])
                    tw2 = ptern.tile([P, M_SEM * P], F32, tag="terntmp", bufs=1)
                    wcf2 = wc2[:].rearrange("p a b -> p (a b)")
                    nc.vector.tensor_scalar(tw2[:], wcf2, sw2_bc, None, ALU.mult)
                    nc.vector.tensor_scalar(tw2[:], tw2[:], 1.49, -1.49, ALU.min,
                                            ALU.max)
                    w2q2 = ptern.tile([P, M_SEM, P], BF16, tag="w1q", bufs=1)
                    nc.vector.tensor_scalar(
                        w2q2[:].rearrange("p a b -> p (a b)"), tw2[:], C_RND,
                        C_RND, ALU.add, ALU.subtract)
                    nc.sync.dma_start(
                        w2q_d[:, m, sub * M_SEM:(sub + 1) * M_SEM], w2q2[:])


        if debug_outs:
            nc.sync.dma_start(dbg["dbg_q"].rearrange("(m p) t -> p m t", p=P), q_sb[:])
            nc.sync.dma_start(dbg["dbg_k"].rearrange("(m p) t -> p m t", p=P), k_sb[:])
            nc.sync.dma_start(dbg["dbg_v"].rearrange("(m p) t -> p m t", p=P), v_sb[:])

        # ====== phases 4-9: token-half pipeline (overlap via per-half deps) ======
        es_so = ExitStack()
        psem = es_so.enter_context(tc.tile_pool(name="psem", bufs=1))
        semout_n = [psem.tile([P, M_SEM, 512], F32, tag=f"so{n}", name=f"so{n}")
                    for n in range(NT_Q)]
        es_opr = ExitStack()
        popr = es_opr.enter_context(tc.tile_pool(name="popr", bufs=1))
        wo_sb = popr.tile([P, M_SEM, DS], BF16)
        nc.sync.dma_start(wo_sb[:], woT.rearrange("(m p) o -> p m o", p=P))

        es_ctx = ExitStack()
        pctx = es_ctx.enter_context(tc.tile_pool(name="pctx", bufs=1))
        ctx_n = [pctx.tile([P, M_SEM, 512], BF16, tag=f"ctx{n}", name=f"ctx{n}")
                 for n in range(NT_Q)]

        with tc.tile_pool(name="pattn", bufs=1) as pattn, \
             tc.tile_pool(name="ps_s", bufs=5, space="PSUM") as ps_s:
            for n in range(NT_Q):
                for h in range(H):
                    pt = pattn.tile([P, NT_K, 512], BF16, tag="ptile", bufs=2)
                    for mt in range(NT_K):
                        ps = ps_s.tile([P, 512], F32, tag="sps")
                        nc.tensor.matmul(ps[:], k_sb[:, h, mt * P:(mt + 1) * P],
                                         q_sb[:, h, n * 512:(n + 1) * 512],
                                         start=True, stop=True)
                        nc.scalar.activation(pt[:, mt, :], ps[:], AF.Exp,
                                             scale=QK_SCALE)
                    td = pattn.tile([P, 8, 512], BF16, tag="dentree", bufs=1)
                    ptf = pt[:].rearrange("p a b -> p (a b)")
                    tdf = td[:].rearrange("p a b -> p (a b)")
                    nc.vector.tensor_tensor(tdf[:, 0:4096], ptf[:, 0:4096],
                                            ptf[:, 4096:8192], op=ALU.add)
                    nc.vector.tensor_tensor(tdf[:, 0:2048], tdf[:, 0:2048],
                                            tdf[:, 2048:4096], op=ALU.add)
                    nc.vector.tensor_tensor(tdf[:, 0:1024], tdf[:, 0:1024],
                                            tdf[:, 1024:2048], op=ALU.add)
                    nc.vector.tensor_tensor(tdf[:, 0:512], tdf[:, 0:512],
                                            tdf[:, 512:1024], op=ALU.add)
                    den_all = pattn.tile([P, 512], F32, tag="denall", bufs=2)
                    nc.gpsimd.partition_all_reduce(den_all[:], td[:, 0, :], P,
                                                   bass_isa.ReduceOp.add)
                    rden_bc = pattn.tile([P, 512], F32, tag="rdenbc", bufs=2)
                    nc.vector.reciprocal_approx_fast(rden_bc[:], den_all[:])
                    cps = ps_mm.tile([P, 512], F32, tag="mm")
                    for mt in range(NT_K):
                        nc.tensor.matmul(cps[:], v_sb[:, mt, h * P:(h + 1) * P],
                                         pt[:, mt, :],
                                         start=(mt == 0), stop=(mt == NT_K - 1))
                    tnorm = pattn.tile([P, 512], F32, tag="ctxnorm", bufs=2)
                    nc.vector.tensor_tensor(tnorm[:], cps[:], rden_bc[:],
                                            op=ALU.mult)
                    nc.vector.tensor_scalar(ctx_n[n][:, h, :], tnorm[:],
                                            bv_sb[:, h:h + 1], None, ALU.add)
        es_qkv.close()

        # ---- out-proj ----
        if True:
            for n in range(NT_Q):
                for m in range(M_SEM):
                    semres = popr.tile([P, 512], F32, tag="semres", bufs=2)
                    nc.sync.dma_start(semres[:],
                                      semT_r[:, m, n * 512:(n + 1) * 512])
                    ps = ps_mm.tile([P, 512], F32, tag="mm")
                    for kk in range(M_SEM):
                        nc.tensor.matmul(ps[:],
                                         wo_sb[:, kk, m * P:(m + 1) * P],
                                         ctx_n[n][:, kk, :],
                                         start=(kk == 0),
                                         stop=(kk == M_SEM - 1))
                    t = popr.tile([P, 512], F32, tag="oproj", bufs=3)
                    nc.scalar.activation(t[:], ps[:], AF.Identity,
                                         bias=bo_sb[:, m:m + 1])
                    nc.vector.tensor_tensor(semout_n[n][:, m, :], t[:],
                                            semres[:], op=ALU.add)
        es_ctx.close()
        es_opr.close()

        # ---- FFN tensors (right side) ----
        es_h = ExitStack()
        ph = es_h.enter_context(tc.tile_pool(name="ph", bufs=1, side="right"))
        h_n = [ph.tile([P, M_FF, 512], BF16, tag=f"h{n}", name=f"h{n}") for n in range(NT_Q)]
        mx2_n = [ph.tile([P, 512], BF16, tag=f"mx2{n}", name=f"mx2{n}") for n in range(NT_Q)]
        mn2_n = [ph.tile([P, 512], BF16, tag=f"mn2{n}", name=f"mn2{n}") for n in range(NT_Q)]
        shbc_n = [ph.tile([P, 512], F32, tag=f"shbc{n}", name=f"shbc{n}") for n in range(NT_Q)]
        dq2_n = [ph.tile([P, 512], F32, tag=f"dq2{n}", name=f"dq2{n}") for n in range(NT_Q)]

        es_xq = ExitStack()
        pxq = es_xq.enter_context(tc.tile_pool(name="pxq", bufs=1,
                                               side="right"))
        xq_n = [pxq.tile([P, M_SEM, 512], BF16, tag=f"xq{n}", name=f"xq{n}")
                for n in range(NT_Q)]
        sxbc_n = [pxq.tile([P, 512], F32, tag=f"sxbc{n}", name=f"sxbc{n}")
                  for n in range(NT_Q)]
        rsxbc_n = [pxq.tile([P, 512], F32, tag=f"rsxbc{n}", name=f"rsxbc{n}")
                   for n in range(NT_Q)]

        # ---- whole FFN complex in ONE scratch scope (no pool barriers) ----
        with tc.tile_pool(name="pffs", bufs=1) as pffs:
            def ffnorm_xquant(n):
                xn = pffs.tile([P, M_SEM, 512], BF16, tag="xn", bufs=1)
                rmsnorm_fm(pffs, lambda m: semout_n[n][:, m, :], M_SEM, 512,
                           gff_sb, xn)
                mx = pffs.tile([P, 512], BF16, tag="bt", bufs=4)
                mn = pffs.tile([P, 512], BF16, tag="bt", bufs=4)
                nc.vector.tensor_tensor(mx[:], xn[:, 0, :], xn[:, 1, :],
                                        op=ALU.max)
                nc.vector.tensor_tensor(mn[:], xn[:, 0, :], xn[:, 1, :],
                                        op=ALU.min)
                for m in range(2, M_SEM):
                    nc.vector.tensor_tensor(mx[:], mx[:], xn[:, m, :],
                                            op=ALU.max)
                    nc.vector.tensor_tensor(mn[:], mn[:], xn[:, m, :],
                                            op=ALU.min)
                am = pffs.tile([P, 512], BF16, tag="bt", bufs=4)
                nc.vector.scalar_tensor_tensor(out=am[:], in0=mn[:],
                                               scalar=-1.0, in1=mx[:],
                                               op0=ALU.mult, op1=ALU.max)
                amc = pffs.tile([P, 512], F32, tag="ft", bufs=2)
                nc.gpsimd.partition_all_reduce(amc[:], am[:], P,
                                               bass_isa.ReduceOp.absmax)
                nc.vector.tensor_scalar(amc[:], amc[:], 1e-5, None, ALU.max)
                nc.vector.reciprocal_approx_fast(sxbc_n[n][:], amc[:])
                nc.vector.tensor_scalar(sxbc_n[n][:], sxbc_n[n][:], 127.0,
                                        None, ALU.mult)
                nc.vector.tensor_scalar(rsxbc_n[n][:], amc[:], 1.0 / 127.0,
                                        None, ALU.mult)
                tq = pffs.tile([P, M_SEM, 512], F32, tag="qtw", bufs=1)
                tqf = tq[:].rearrange("p a b -> p (a b)")
                nc.vector.tensor_tensor(tq[:], xn[:],
                                        bcast_free(sxbc_n[n][:], M_SEM),
                                        op=ALU.mult)
                nc.vector.tensor_scalar(tqf[:], tqf[:], C_RND, C_RND, ALU.add,
                                        ALU.subtract)
                nc.vector.tensor_tensor(xq_n[n][:], tq[:],
                                        bcast_free(rsxbc_n[n][:], M_SEM),
                                        op=ALU.mult)

            def ffn1(n):
                for m in range(M_FF):
                    w1q = pffs.tile([P, M_FF, P], BF16, tag="wq", bufs=2)
                    nc.sync.dma_start(w1q[:, :M_SEM, :], w1q_d[:, m])
                    ps = ps_mm.tile([P, 512], F32, tag="mm")
                    for kk in range(M_SEM):
                        nc.tensor.matmul(ps[:], w1q[:, kk, :],
                                         xq_n[n][:, kk, :],
                                         start=(kk == 0),
                                         stop=(kk == M_SEM - 1))
                    sn = pffs.tile([P, 512], BF16, tag="bt", bufs=4)
                    nc.scalar.activation(sn[:], ps[:], AF.Sin,
                                         scale=alphap[:, m:m + 1])
                    sq2 = pffs.tile([P, 512], BF16, tag="bt", bufs=4)
                    nc.scalar.activation(sq2[:], sn[:], AF.Square)
                    nc.vector.scalar_tensor_tensor(
                        out=h_n[n][:, m, :], in0=sq2[:],
                        scalar=rbetap[:, m:m + 1], in1=ps[:],
                        op0=ALU.mult, op1=ALU.add)
                    if m == 0:
                        nc.vector.tensor_copy(mx2_n[n][:], h_n[n][:, 0, :])
                        nc.vector.tensor_copy(mn2_n[n][:], h_n[n][:, 0, :])
                    else:
                        nc.vector.tensor_tensor(mx2_n[n][:], mx2_n[n][:],
                                                h_n[n][:, m, :], op=ALU.max)
                        nc.vector.tensor_tensor(mn2_n[n][:], mn2_n[n][:],
                                                h_n[n][:, m, :], op=ALU.min)

            def hquant(n):
                am2 = pffs.tile([P, 512], BF16, tag="bt", bufs=4)
                nc.vector.scalar_tensor_tensor(out=am2[:], in0=mn2_n[n][:],
                                               scalar=-1.0, in1=mx2_n[n][:],
                                               op0=ALU.mult, op1=ALU.max)
                amc2 = pffs.tile([P, 512], F32, tag="ft", bufs=2)
                nc.gpsimd.partition_all_reduce(amc2[:], am2[:], P,
                                               bass_isa.ReduceOp.absmax)
                nc.vector.tensor_scalar(amc2[:], amc2[:], mw1, 1e-5, ALU.mult,
                                        ALU.max)
                nc.vector.reciprocal_approx_fast(shbc_n[n][:], amc2[:])
                nc.vector.tensor_scalar(shbc_n[n][:], shbc_n[n][:], mw1, 127.0,
                                        ALU.mult, ALU.mult)
                nc.vector.tensor_scalar(dq2_n[n][:], amc2[:], mw2, 1.0 / 127.0,
                                        ALU.mult, ALU.mult)
                for c4 in range(M_FF // M_SEM):
                    tq2 = pffs.tile([P, M_SEM, 512], F32, tag="qtw", bufs=1)
                    tq2f = tq2[:].rearrange("p a b -> p (a b)")
                    nc.vector.tensor_tensor(
                        tq2[:], h_n[n][:, c4 * M_SEM:(c4 + 1) * M_SEM, :],
                        bcast_free(shbc_n[n][:], M_SEM), op=ALU.mult)
                    nc.vector.tensor_scalar(
                        h_n[n][:, c4 * M_SEM:(c4 + 1) * M_SEM, :]
                        .rearrange("p a b -> p (a b)"),
                        tq2f[:], C_RND, C_RND, ALU.add, ALU.subtract)

            def ffn2(n):
                for m in range(M_SEM):
                    w2q = pffs.tile([P, M_FF, P], BF16, tag="wq", bufs=2)
                    nc.sync.dma_start(w2q[:], w2q_d[:, m])
                    ps = ps_mm.tile([P, 512], F32, tag="mm")
                    for kk in range(M_FF):
                        nc.tensor.matmul(ps[:], w2q[:, kk, :], h_n[n][:, kk, :],
                                         start=(kk == 0),
                                         stop=(kk == M_FF - 1))
                    t = pffs.tile([P, 512], F32, tag="qt", bufs=3)
                    nc.vector.tensor_tensor(t[:], ps[:], dq2_n[n][:],
                                            op=ALU.mult)
                    yo = pffs.tile([P, 512], F32, tag="qt", bufs=3)
                    nc.vector.tensor_tensor(yo[:], t[:], semout_n[n][:, m, :],
                                            op=ALU.add)
                    nc.sync.dma_start(outT[m * P:(m + 1) * P,
                                           n * 512:(n + 1) * 512], yo[:])

            ffnorm_xquant(0)
            ffnorm_xquant(1)
            ffn1(0)
            ffn1(1)
            hquant(0)
            hquant(1)
            ffn2(0)
            ffn2(1)
        es_xq.close()
        es_h.close()
        es_so.close()

    nc.compile()
    return nc


_NC_CACHE = {}


def _get_nc(debug_outs=False):
    key = bool(debug_outs)
    if key not in _NC_CACHE:
        _NC_CACHE[key] = build_nc(debug_outs)
    return _NC_CACHE[key]


def make_in_maps(inputs):
    """Host-side shard + layout prep. inputs: dict of full np arrays."""
    import ml_dtypes
    bf = ml_dtypes.bfloat16
    f32 = np.float32
    sem = np.asarray(inputs["sem"], f32)
    pro = np.asarray(inputs["pro"], f32)

    def cols(v, nm):
        return np.ascontiguousarray(np.asarray(v, f32).reshape(nm, P).T)

    common = {
        "gsem": cols(inputs["g_sem"], M_SEM),
        "gpro": cols(inputs["g_pro"], M_PRO),
        "gff": cols(inputs["g_ff"], M_SEM),
        "bq": cols(inputs["bq"], M_SEM),
        "bk": cols(inputs["bk"], M_SEM),
        "bv": cols(inputs["bv"], M_SEM),
        "bo": cols(inputs["bo"], M_SEM),
        "alpha": cols(inputs["alpha"], M_FF),
        "beta": cols(inputs["beta"], M_FF),
        "w1T": np.ascontiguousarray(np.asarray(inputs["W1"], f32).T),
        "w2T": np.ascontiguousarray(np.asarray(inputs["W2"], f32).T),
        "wqT": np.ascontiguousarray(np.asarray(inputs["Wq"], f32).T).astype(bf),
        "wkT": np.ascontiguousarray(np.asarray(inputs["Wk"], f32).T).astype(bf),
        "wvT": np.ascontiguousarray(np.asarray(inputs["Wv"], f32).T).astype(bf),
        "woT": np.ascontiguousarray(np.asarray(inputs["Wo"], f32).T).astype(bf),
    }

    in_maps = []
    for c in range(N_CORES):
        b, half = c // 2, c % 2
        m = dict(common)
        m["semT"] = np.ascontiguousarray(sem[b, half * TOK:(half + 1) * TOK, :].T)
        m["proT"] = np.ascontiguousarray(pro[b].T)
        m["w1s"] = np.ascontiguousarray(common["w1T"][c * P:(c + 1) * P, :])
        m["w2s"] = np.ascontiguousarray(common["w2T"][c * DP:(c + 1) * DP, :])
        in_maps.append(m)
    return in_maps


def assemble_out(results):
    out = np.empty((B, S, DS), np.float32)
    for c in range(N_CORES):
        b, half = c // 2, c % 2
        out[b, half * TOK:(half + 1) * TOK, :] = results[c]["outT"].T
    return out


def kernel(**inputs):
    nc = _get_nc()
    in_maps = make_in_maps(inputs)
    res = run_bass_kernel_spmd(nc, in_maps, core_ids=list(range(N_CORES)))
    return assemble_out(res.results)



# revision 10
# speedup vs baseline: 1.1162x; 1.1162x over previous
"""Trainium2 Bass kernel for nn_CrossAttentionFusion (cross-attention + BitLinear FFN).

Sharding: 8 cores = 4 batches x 2 sequence-halves. Each core owns 1024 query
tokens; K/V computed over the batch's full 2048 tokens. Ternarize of W1/W2 is
sharded 8-way and AllGathered as fp8. Attention runs fully in fp8 (DoubleRow
for K>=256 contractions); FFN runs ternary-fp8 x bf16 with activation
quantization folded out (bf16 storage sits inside the reference's int8
quantization noise band). Softmax denominators ride the PE (ones-matmul reduce
+ K=1 broadcast matmul) instead of gpsimd.
"""
import math
import numpy as np
from contextlib import ExitStack

import concourse.bass as bass
import concourse.tile as tile
from concourse import bacc, mybir
from concourse.bass_utils import run_bass_kernel_spmd

F32 = mybir.dt.float32
BF16 = mybir.dt.bfloat16
FP8 = mybir.dt.float8e4
AF = mybir.ActivationFunctionType
ALU = mybir.AluOpType
DR = mybir.MatmulPerfMode.DoubleRow

B, S, DS, DP, H = 4, 2048, 1024, 512, 8
DF = 4 * DS
HD = DS // H          # 128
TOK = 1024            # query tokens per core
N_CORES = 8
EPS = 1e-6
C_RND = 12582912.0    # 1.5 * 2**23 : +C-C = round-to-nearest-even
QS_UP = 16.0          # fp8 range lift for q
QK_SCALE = QS_UP / math.sqrt(HD)
EXP_SCALE = 1.0 / QS_UP

P = 128
MS = DS // P          # 8  k-tiles of sem/ds
MP = DP // P          # 4  k-tiles of pro
MF = DF // P          # 32 k-tiles of ffn hidden
NT = TOK // 512       # 2  query chunks
NK = S // P           # 16 key tiles
DFS = DF // N_CORES   # 512  w1 shard cols
DSS = DS // N_CORES   # 128  w2 shard cols


def bcast_part(ap2d, rep):
    """[1, W] AP -> [rep, W] AP with partition step 0 (DRAM broadcast)."""
    return bass.AP(tensor=ap2d.tensor, offset=ap2d.offset,
                   ap=[[0, rep]] + list(ap2d.ap[1:]))


def build_nc(debug_outs=False):
    nc = bacc.Bacc("TRN2", target_bir_lowering=False, debug=False,
                   num_devices=N_CORES)

    semT = nc.dram_tensor("semT", [DS, TOK], F32, kind="ExternalInput").ap()
    proT = nc.dram_tensor("proT", [DP, S], F32, kind="ExternalInput").ap()
    wqT = nc.dram_tensor("wqT", [DS, DS], FP8, kind="ExternalInput").ap()
    wkT = nc.dram_tensor("wkT", [DP, DS], FP8, kind="ExternalInput").ap()
    wvT = nc.dram_tensor("wvT", [DP, DS], FP8, kind="ExternalInput").ap()
    woT = nc.dram_tensor("woT", [DS, DS], FP8, kind="ExternalInput").ap()
    w1shT = nc.dram_tensor("w1shT", [DS, DFS], F32, kind="ExternalInput").ap()
    w2shT = nc.dram_tensor("w2shT", [DF, DSS], F32, kind="ExternalInput").ap()
    bvrow = nc.dram_tensor("bvrow", [1, DS], BF16, kind="ExternalInput").ap()
    gsem = nc.dram_tensor("gsem", [P, MS], F32, kind="ExternalInput").ap()
    gpro = nc.dram_tensor("gpro", [P, MP], F32, kind="ExternalInput").ap()
    gff = nc.dram_tensor("gff", [P, MS], F32, kind="ExternalInput").ap()
    bq = nc.dram_tensor("bq", [P, MS], F32, kind="ExternalInput").ap()
    bk = nc.dram_tensor("bk", [P, MS], F32, kind="ExternalInput").ap()
    bo = nc.dram_tensor("bo", [P, MS], F32, kind="ExternalInput").ap()
    alpha = nc.dram_tensor("alpha", [P, MF], F32, kind="ExternalInput").ap()
    beta = nc.dram_tensor("beta", [P, MF], F32, kind="ExternalInput").ap()
    outT = nc.dram_tensor("outT", [DS, TOK], F32, kind="ExternalOutput").ap()

    dbg = {}
    if debug_outs:
        for name, shape, dt in [
            ("dbg_semn", [DS, TOK], FP8), ("dbg_pron", [DP, S], FP8),
            ("dbg_q", [DS, TOK], FP8), ("dbg_k", [DS, S], FP8),
            ("dbg_v", [S, DS], FP8), ("dbg_ctx", [DS, TOK], FP8),
            ("dbg_semout", [DS, TOK], F32), ("dbg_xn", [DS, TOK], BF16),
            ("dbg_h", [DF, TOK], BF16), ("dbg_mw", [P, 2], F32),
        ]:
            dbg[name] = nc.dram_tensor(name, shape, dt, kind="ExternalOutput").ap()

    with tile.TileContext(nc) as tc, ExitStack() as top:
        persist = top.enter_context(tc.tile_pool(name="persist", bufs=1))
        ps_small = top.enter_context(tc.tile_pool(name="ps_small", bufs=2,
                                                  space="PSUM"))
        pdram = top.enter_context(tc.tile_pool(name="pdram", bufs=1,
                                               space="DRAM"))

        # ---------------- constants & params ----------------
        ones_col = persist.tile([P, 1], BF16)
        nc.vector.memset(ones_col[:], 1.0)
        ones_colf = persist.tile([P, 1], F32)
        nc.vector.memset(ones_colf[:], 1.0)
        ones_row = persist.tile([1, P], F32)
        nc.vector.memset(ones_row[:], 1.0)
        ones_rowb = persist.tile([1, P], BF16)
        nc.vector.memset(ones_rowb[:], 1.0)
        ones2_32 = persist.tile([P, 2, 32], FP8)
        nc.vector.memset(ones2_32[:], 1.0)
        eps_t = persist.tile([1, 1], F32)
        nc.vector.memset(eps_t[:], EPS)

        gsem_sb = persist.tile([P, MS], F32)
        gpro_sb = persist.tile([P, MP], F32)
        gff_sb = persist.tile([P, MS], F32)
        bq_sb = persist.tile([P, MS], F32)
        bk_sb = persist.tile([P, MS], F32)
        bo_sb = persist.tile([P, MS], F32)
        alpha_sb = persist.tile([P, MF], F32)
        beta_sb = persist.tile([P, MF], F32)
        bvrow_sb = persist.tile([1, DS], BF16)
        for ap_d, t in [(gsem, gsem_sb), (gpro, gpro_sb), (gff, gff_sb),
                        (bq, bq_sb), (bk, bk_sb), (bo, bo_sb),
                        (alpha, alpha_sb), (beta, beta_sb),
                        (bvrow, bvrow_sb)]:
            nc.sync.dma_start(t[:], ap_d[:])

        # scalars derived from the weight-mean AllReduce (filled in later)
        m_bc = persist.tile([P, 2], F32)
        s_bc = persist.tile([P, 2], F32)
        alphap = persist.tile([P, MF], F32)
        rbetap = persist.tile([P, MF], F32)
        dq_col = persist.tile([P, 1], F32)

        # collective DRAM tensors
        ccm_in = pdram.tile([1, 2], F32)
        ccm_out = pdram.tile([1, 2], F32, addr_space="Shared")
        cin1 = pdram.tile([DS * DFS], FP8)
        w1g = pdram.tile([N_CORES * DS * DFS], FP8, addr_space="Shared")
        cin2 = pdram.tile([DF * DSS], FP8)
        w2g = pdram.tile([N_CORES * DF * DSS], FP8, addr_space="Shared")

        # ======== phase W part 1: abs-mean of W shards + AllReduce ========
        esw = ExitStack()
        pw = esw.enter_context(tc.tile_pool(name="pw", bufs=1))
        w1s_sb = pw.tile([P, MS, DFS], F32)
        w1l = pw.tile([P, MS, DFS], FP8)
        esw2 = ExitStack()
        pw2 = esw2.enter_context(tc.tile_pool(name="pw2", bufs=1))
        w2s_sb = pw2.tile([P, MF, DSS], F32)
        w2l = pw2.tile([P, MF, DSS], FP8)
        nc.sync.dma_start(w1s_sb[:], w1shT.rearrange("(m p) c -> p m c", p=P))
        nc.sync.dma_start(w2s_sb[:], w2shT.rearrange("(m p) c -> p m c", p=P))

        acc1c = persist.tile([P, MS], F32)
        acc2c = persist.tile([P, MS], F32)
        for kt in range(MS):
            ab = pw2.tile([P, DFS], BF16, tag="ab", bufs=2)
            nc.scalar.activation(ab[:], w1s_sb[:, kt, :], AF.Abs,
                                 accum_out=acc1c[:, kt:kt + 1])
        for kt in range(MS):
            ab = pw2.tile([P, DFS], BF16, tag="ab", bufs=2)
            nc.scalar.activation(
                ab[:, 0:4 * DSS],
                w2s_sb[:, 4 * kt:4 * kt + 4, :].rearrange("p a b -> p (a b)"),
                AF.Abs, accum_out=acc2c[:, kt:kt + 1])
        acc1 = persist.tile([P, 1], F32)
        acc2 = persist.tile([P, 1], F32)
        nc.vector.tensor_reduce(acc1[:], acc1c[:], axis=mybir.AxisListType.X,
                                op=ALU.add)
        nc.vector.tensor_reduce(acc2[:], acc2c[:], axis=mybir.AxisListType.X,
                                op=ALU.add)

        ps_w = ps_small.tile([1, 2], F32, tag="mm")
        nc.tensor.matmul(ps_w[:, 0:1], ones_colf[:], acc1[:],
                         start=True, stop=True)
        nc.tensor.matmul(ps_w[:, 1:2], ones_colf[:], acc2[:],
                         start=True, stop=True)
        loc = persist.tile([1, 2], F32)
        nc.vector.tensor_copy(loc[:], ps_w[:])
        nc.sync.dma_start(ccm_in[:], loc[:])
        nc.gpsimd.collective_compute(
            "AllReduce", ALU.add, replica_groups=[list(range(N_CORES))],
            ins=[ccm_in.opt()], outs=[ccm_out.opt()])
        # broadcast the 2 sums to all partitions straight from DRAM
        m_raw = persist.tile([P, 2], F32)
        nc.sync.dma_start(m_raw[:], bcast_part(ccm_out[:], P))

        # ================ phase N: input norms + QKV projections ================
        esn = ExitStack()
        patt = esn.enter_context(tc.tile_pool(name="patt", bufs=1,
                                              side="right"))
        q_sb = patt.tile([P, MS, TOK], FP8)
        k_sb = patt.tile([P, MS, S], FP8)
        v_sb = patt.tile([P, NK, DS], FP8)
        ctx_sb = patt.tile([P, NT, MS, 512], FP8)
        wo_sb = patt.tile([P, MS, DS], FP8)
        nc.sync.dma_start(wo_sb[:], woT.rearrange("(m p) o -> p m o", p=P))

        with tc.tile_pool(name="pnorm", bufs=1) as pnorm:
            semT_sb = pnorm.tile([P, MS, TOK], F32)
            nc.sync.dma_start(semT_sb[:],
                              semT.rearrange("(m p) t -> p m t", p=P))
            semn_sb = pnorm.tile([P, MS, TOK], FP8)
            pron_sb = pnorm.tile([P, MP, S], FP8)
            wq_sb = pnorm.tile([P, MS, DS], FP8)
            wk_sb = pnorm.tile([P, MP, DS], FP8)
            wv_sb = pnorm.tile([P, MP, DS], FP8)
            nc.sync.dma_start(wk_sb[:], wkT.rearrange("(m p) o -> p m o", p=P))
            nc.sync.dma_start(wv_sb[:], wvT.rearrange("(m p) o -> p m o", p=P))
            nc.sync.dma_start(wq_sb[:], wqT.rearrange("(m p) o -> p m o", p=P))

            def rmsnorm(fetch, nm, D, T, g_sb, out_sb):
                for ch in range(T // 512):
                    cs = slice(ch * 512, (ch + 1) * 512)
                    x_ch = fetch(ch)    # [P, nm, 512]
                    sq = pnorm.tile([P, 8, 512], BF16, tag="sq", bufs=1)
                    nc.scalar.activation(sq[:, :nm, :], x_ch, AF.Square)
                    msp = ps_small.tile([1, 512], F32, tag="mm")
                    for kt in range(nm):
                        nc.tensor.matmul(msp[:], ones_col[:], sq[:, kt, :],
                                         start=(kt == 0), stop=(kt == nm - 1))
                    lnr = pnorm.tile([1, 512], F32, tag="lnr", bufs=2)
                    nc.scalar.activation(lnr[:], msp[:], AF.Ln,
                                         bias=eps_t[:], scale=1.0 / D)
                    rs_row = pnorm.tile([1, 512], F32, tag="rs", bufs=2)
                    nc.scalar.activation(rs_row[:], lnr[:], AF.Exp, scale=-0.5)
                    rsb = ps_small.tile([P, 512], F32, tag="mm")
                    nc.tensor.matmul(rsb[:], ones_row[:], rs_row[:],
                                     start=True, stop=True)
                    for kt in range(nm):
                        nc.vector.scalar_tensor_tensor(
                            out=out_sb[:, kt, cs], in0=x_ch[:, kt, :],
                            scalar=g_sb[:, kt:kt + 1], in1=rsb[:],
                            op0=ALU.mult, op1=ALU.mult)

            def pro_fetch(ch):
                t = pnorm.tile([P, MP, 512], F32, tag="proT", bufs=2,
                               name="proT_ch")
                nc.sync.dma_start(
                    t[:], proT.rearrange("(m p) t -> p m t", p=P)
                    [:, :, ch * 512:(ch + 1) * 512])
                return t[:]

            rmsnorm(pro_fetch, MP, DP, S, gpro_sb, pron_sb)
            rmsnorm(lambda ch: semT_sb[:, :, ch * 512:(ch + 1) * 512],
                    MS, DS, TOK, gsem_sb, semn_sb)

            # ---- K projection: k[ds_feat, key_tok]
            for ch in range(S // 512):
                cs = slice(ch * 512, (ch + 1) * 512)
                for m in range(MS):
                    ps = ps_small.tile([P, 512], F32, tag="mm")
                    for j in range(MP // 2):
                        nc.tensor.matmul(
                            ps[:], wk_sb[:, 2 * j:2 * j + 2,
                                         m * P:(m + 1) * P],
                            pron_sb[:, 2 * j:2 * j + 2, cs],
                            start=(j == 0), stop=(j == MP // 2 - 1),
                            perf_mode=DR)
                    nc.vector.tensor_scalar(k_sb[:, m, cs], ps[:],
                                            bk_sb[:, m:m + 1], None, ALU.add)

            # ---- Q projection (pre-scaled by QK_SCALE)
            for n in range(NT):
                cs = slice(n * 512, (n + 1) * 512)
                for m in range(MS):
                    ps = ps_small.tile([P, 512], F32, tag="mm")
                    for j in range(MS // 2):
                        nc.tensor.matmul(
                            ps[:], wq_sb[:, 2 * j:2 * j + 2,
                                         m * P:(m + 1) * P],
                            semn_sb[:, 2 * j:2 * j + 2, cs],
                            start=(j == 0), stop=(j == MS // 2 - 1),
                            perf_mode=DR)
                    nc.vector.tensor_scalar(q_sb[:, m, cs], ps[:],
                                            bq_sb[:, m:m + 1], QK_SCALE,
                                            ALU.add, ALU.mult)

            # ---- V projection (+bv folded in via ones-row matmul)
            for mt in range(NK):
                for dc in range(DS // 512):
                    ds_sl = slice(dc * 512, (dc + 1) * 512)
                    ps = ps_small.tile([P, 512], F32, tag="mm")
                    for j in range(MP // 2):
                        nc.tensor.matmul(
                            ps[:], pron_sb[:, 2 * j:2 * j + 2,
                                           mt * P:(mt + 1) * P],
                            wv_sb[:, 2 * j:2 * j + 2, ds_sl],
                            start=(j == 0), stop=False, perf_mode=DR)
                    nc.tensor.matmul(ps[:], ones_rowb[:], bvrow_sb[:, ds_sl],
                                     start=False, stop=True)
                    nc.scalar.activation(v_sb[:, mt, ds_sl], ps[:], AF.Copy)

            # ---- post-AllReduce scalar chain (vector; emitted after the
            #      phase-N vector work so the queue doesn't stall on the cc)
            nc.vector.tensor_scalar(m_bc[:], m_raw[:], 1.0 / (DS * DF), None,
                                    ALU.mult)
            nc.vector.reciprocal(s_bc[:], m_bc[:])
            nc.vector.tensor_scalar(alphap[:], alpha_sb[:], m_bc[:, 0:1],
                                    None, ALU.mult)
            nc.vector.tensor_scalar(rbetap[:], beta_sb[:], 1e-9, None,
                                    ALU.add)
            nc.vector.reciprocal(rbetap[:], rbetap[:])
            nc.vector.tensor_scalar(rbetap[:], rbetap[:], s_bc[:, 0:1], None,
                                    ALU.mult)
            nc.vector.tensor_tensor(dq_col[:], m_bc[:, 0:1], m_bc[:, 1:2],
                                    op=ALU.mult)
            if debug_outs:
                nc.sync.dma_start(dbg["dbg_mw"][:], m_bc[:])

            # ---- W1 ternarize (vector), W2 ternarize (gpsimd, after evacs)
            for kt in range(MS):
                t1 = pnorm.tile([P, DFS], F32, tag="w1t1", bufs=2)
                nc.vector.tensor_scalar(t1[:], w1s_sb[:, kt, :],
                                        s_bc[:, 0:1], 1.49, ALU.mult, ALU.min)
                nc.vector.tensor_scalar(t1[:], t1[:], -1.49, C_RND, ALU.max,
                                        ALU.add)
                nc.vector.tensor_scalar(w1l[:, kt, :], t1[:], C_RND, None,
                                        ALU.subtract)
            nc.sync.dma_start(
                cin1[:].rearrange("(m p c) -> p m c", p=P, m=MS, c=DFS),
                w1l[:])
            nc.gpsimd.collective_compute(
                "AllGather", ALU.bypass, replica_groups=[list(range(N_CORES))],
                ins=[cin1.opt()], outs=[w1g.opt()])

            for kt in range(MF // 4):
                fs = slice(kt * 4, (kt + 1) * 4)
                t2 = pnorm.tile([P, 4, DSS], F32, tag="w2t1", bufs=2)
                t2f = t2[:].rearrange("p a b -> p (a b)")
                nc.gpsimd.tensor_scalar(t2f, w2s_sb[:, fs, :]
                                        .rearrange("p a b -> p (a b)"),
                                        s_bc[:, 1:2], 1.49, ALU.mult, ALU.min)
                nc.gpsimd.tensor_scalar(t2f, t2f, -1.49, C_RND, ALU.max,
                                        ALU.add)
                nc.gpsimd.tensor_scalar(w2l[:, fs, :]
                                        .rearrange("p a b -> p (a b)"),
                                        t2f, C_RND, None, ALU.subtract)
            nc.sync.dma_start(
                cin2[:].rearrange("(m p c) -> p m c", p=P, m=MF, c=DSS),
                w2l[:])
            nc.gpsimd.collective_compute(
                "AllGather", ALU.bypass, replica_groups=[list(range(N_CORES))],
                ins=[cin2.opt()], outs=[w2g.opt()])

            if debug_outs:
                nc.sync.dma_start(dbg["dbg_semn"]
                                  .rearrange("(m p) t -> p m t", p=P),
                                  semn_sb[:])
                nc.sync.dma_start(dbg["dbg_pron"]
                                  .rearrange("(m p) t -> p m t", p=P),
                                  pron_sb[:])
        esw2.close()

        if debug_outs:
            nc.sync.dma_start(dbg["dbg_q"].rearrange("(m p) t -> p m t", p=P),
                              q_sb[:])
            nc.sync.dma_start(dbg["dbg_k"].rearrange("(m p) t -> p m t", p=P),
                              k_sb[:])
            nc.sync.dma_start(dbg["dbg_v"].rearrange("(t p) d -> p t d", p=P),
                              v_sb[:])

        esw.close()   # w1s f32 freed

        # ================ phase A: attention ================
        with tc.tile_pool(name="pa", bufs=1) as pa, \
             tc.tile_pool(name="ps_att", bufs=1, space="PSUM") as ps_att:
            for h in range(H):
                for n in range(NT):
                    qs = slice(n * 512, (n + 1) * 512)
                    pt = pa.tile([P, NK, 512], FP8, tag="pt", bufs=2)
                    for g in range(8):
                        scp = ps_att.tile([P, 1024], F32, tag="sc", bufs=2)
                        for mi in range(2):
                            mt = 2 * g + mi
                            nc.tensor.matmul(
                                scp[:, mi * 512:(mi + 1) * 512],
                                k_sb[:, h, mt * P:(mt + 1) * P],
                                q_sb[:, h, qs], start=True, stop=True)
                        nc.scalar.activation(
                            pt[:, 2 * g:2 * g + 2, :], scp[:], AF.Exp,
                            scale=EXP_SCALE)
                    dnp = ps_att.tile([P, 512], F32, tag="dr", bufs=1)
                    for j in range(NK // 2):
                        nc.tensor.matmul(
                            dnp[0:32, :], ones2_32[:],
                            pt[:, 2 * j:2 * j + 2, :],
                            start=(j == 0), stop=(j == NK // 2 - 1),
                            perf_mode=DR)
                    rden = pa.tile([1, 512], F32, tag="rden", bufs=2)
                    nc.vector.reciprocal_approx_fast(rden[:], dnp[0:1, :])
                    rdb = ps_att.tile([P, 512], F32, tag="dr", bufs=1)
                    nc.tensor.matmul(rdb[:], ones_row[:], rden[:],
                                     start=True, stop=True)
                    rdb_sb = pa.tile([P, 512], BF16, tag="rdbs", bufs=2)
                    nc.vector.tensor_copy(rdb_sb[:], rdb[:])
                    ctxp = ps_att.tile([P, 512], F32, tag="ctx", bufs=1)
                    for j in range(NK // 2):
                        nc.tensor.matmul(
                            ctxp[:], v_sb[:, 2 * j:2 * j + 2,
                                          h * P:(h + 1) * P],
                            pt[:, 2 * j:2 * j + 2, :],
                            start=(j == 0), stop=(j == NK // 2 - 1),
                            perf_mode=DR)
                    nc.vector.tensor_tensor(ctx_sb[:, n, h, :], ctxp[:],
                                            rdb_sb[:], op=ALU.mult)

        if debug_outs:
            nc.sync.dma_start(dbg["dbg_ctx"]
                              .rearrange("(m p) (n c) -> p n m c", p=P, n=NT),
                              ctx_sb[:])

        # ================ phase O: out-projection + residual ================
        eso = ExitStack()
        po = eso.enter_context(tc.tile_pool(name="po", bufs=1))
        semout = po.tile([P, NT, MS, 512], F32)
        for n in range(NT):
            for m in range(MS):
                semres = po.tile([P, 512], F32, tag="semres", bufs=3)
                nc.sync.dma_start(
                    semres[:], semT[m * P:(m + 1) * P,
                                    n * 512:(n + 1) * 512])
                ps = ps_small.tile([P, 512], F32, tag="mm")
                for j in range(MS // 2):
                    nc.tensor.matmul(
                        ps[:], wo_sb[:, 2 * j:2 * j + 2, m * P:(m + 1) * P],
                        ctx_sb[:, n, 2 * j:2 * j + 2, :],
                        start=(j == 0), stop=(j == MS // 2 - 1), perf_mode=DR)
                nc.vector.scalar_tensor_tensor(
                    out=semout[:, n, m, :], in0=ps[:],
                    scalar=bo_sb[:, m:m + 1], in1=semres[:],
                    op0=ALU.add, op1=ALU.add)
        esn.close()   # q/k/v/ctx/wo freed

        if debug_outs:
            nc.sync.dma_start(dbg["dbg_semout"]
                              .rearrange("(m p) (n c) -> p n m c", p=P, n=NT),
                              semout[:])

        # ================ phase F: BitLinear FFN ================
        with tc.tile_pool(name="pf", bufs=1) as pf, \
             tc.tile_pool(name="ps_f", bufs=1, space="PSUM") as ps_f:
            xns = []
            for n in range(NT):
                xn = pf.tile([P, MS, 512], BF16, tag="xn", bufs=2, name="xn")
                sq = pf.tile([P, MS, 512], BF16, tag="sq", bufs=1, name="sq")
                nc.scalar.activation(sq[:], semout[:, n], AF.Square)
                msp = ps_f.tile([1, 512], F32, tag="nrm", bufs=1)
                for kt in range(MS):
                    nc.tensor.matmul(msp[:], ones_col[:], sq[:, kt, :],
                                     start=(kt == 0), stop=(kt == MS - 1))
                lnr = pf.tile([1, 512], F32, tag="lnr", bufs=2, name="lnr")
                nc.scalar.activation(lnr[:], msp[:], AF.Ln, bias=eps_t[:],
                                     scale=1.0 / DS)
                rs_row = pf.tile([1, 512], F32, tag="rs", bufs=2, name="rs_row")
                nc.scalar.activation(rs_row[:], lnr[:], AF.Exp, scale=-0.5)
                rsb = ps_f.tile([P, 512], F32, tag="nrm2", bufs=1)
                nc.tensor.matmul(rsb[:], ones_row[:], rs_row[:], start=True,
                                 stop=True)
                for kt in range(MS):
                    nc.vector.scalar_tensor_tensor(
                        out=xn[:, kt, :], in0=semout[:, n, kt, :],
                        scalar=gff_sb[:, kt:kt + 1], in1=rsb[:],
                        op0=ALU.mult, op1=ALU.mult)
                xns.append(xn)
                if debug_outs:
                    nc.sync.dma_start(
                        dbg["dbg_xn"].rearrange("(m p) t -> p m t", p=P)
                        [:, :, n * 512:(n + 1) * 512], xn[:])

            # --- FFN1 + snake -> h' = h2/mw1 (bf16); weights streamed
            hs = [pf.tile([P, MF, 512], BF16, tag=f"h{n}", name=f"h{n}")
                  for n in range(NT)]
            for g in range(MF // 2):
                c, off = (2 * g * P) // DFS, (2 * g * P) % DFS
                w1w = pf.tile([P, MS, 2 * P], FP8, tag="w1w", bufs=3,
                              name="w1w")
                nc.sync.dma_start(
                    w1w[:],
                    w1g[c * DS * DFS:(c + 1) * DS * DFS]
                    .rearrange("(m p f) -> p m f", p=P, m=MS, f=DFS)
                    [:, :, off:off + 2 * P])
                for n in range(NT):
                    fps = ps_f.tile([P, 1024], F32, tag="f1", bufs=2)
                    sn = pf.tile([P, 2, 512], BF16, tag="sn", bufs=2,
                                 name="sn")
                    sq2 = pf.tile([P, 2, 512], BF16, tag="sq2", bufs=2,
                                  name="sq2")
                    for mi in range(2):
                        m = 2 * g + mi
                        for kt in range(MS):
                            nc.tensor.matmul(
                                fps[:, mi * 512:(mi + 1) * 512],
                                w1w[:, kt, mi * P:(mi + 1) * P],
                                xns[n][:, kt, :],
                                start=(kt == 0), stop=(kt == MS - 1))
                        nc.scalar.activation(sn[:, mi, :],
                                             fps[:, mi * 512:(mi + 1) * 512],
                                             AF.Sin,
                                             scale=alphap[:, m:m + 1])
                    nc.vector.tensor_tensor(
                        sq2[:].rearrange("p a b -> p (a b)"),
                        sn[:].rearrange("p a b -> p (a b)"),
                        sn[:].rearrange("p a b -> p (a b)"), op=ALU.mult)
                    for mi in range(2):
                        m = 2 * g + mi
                        nc.vector.scalar_tensor_tensor(
                            out=hs[n][:, m, :], in0=sq2[:, mi, :],
                            scalar=rbetap[:, m:m + 1],
                            in1=fps[:, mi * 512:(mi + 1) * 512],
                            op0=ALU.mult, op1=ALU.add)
            if debug_outs:
                for n in range(NT):
                    nc.sync.dma_start(
                        dbg["dbg_h"].rearrange("(m p) t -> p m t", p=P)
                        [:, :, n * 512:(n + 1) * 512], hs[n][:])

            # --- FFN2 + dequant + residual -> outT; weights streamed
            for m in range(MS):
                w2w = pf.tile([P, MF, DSS], FP8, tag="w2w", bufs=2,
                              name="w2w")
                nc.sync.dma_start(
                    w2w[:],
                    w2g[m * DF * DSS:(m + 1) * DF * DSS]
                    .rearrange("(k p f) -> p k f", p=P, k=MF, f=DSS))
                for n in range(NT):
                    ps2 = ps_small.tile([P, 512], F32, tag="mm")
                    for kt in range(MF):
                        nc.tensor.matmul(
                            ps2[:], w2w[:, kt, :], hs[n][:, kt, :],
                            start=(kt == 0), stop=(kt == MF - 1))
                    yo = pf.tile([P, 512], F32, tag="yo", bufs=3, name="yo")
                    nc.vector.scalar_tensor_tensor(
                        out=yo[:], in0=ps2[:], scalar=dq_col[:],
                        in1=semout[:, n, m, :], op0=ALU.mult, op1=ALU.add)
                    nc.sync.dma_start(outT[m * P:(m + 1) * P,
                                           n * 512:(n + 1) * 512], yo[:])
        eso.close()

    nc.compile()
    return nc


_NC_CACHE = {}


def _get_nc(debug_outs=False):
    key = bool(debug_outs)
    if key not in _NC_CACHE:
        _NC_CACHE[key] = build_nc(debug_outs)
    return _NC_CACHE[key]


def make_in_maps(inputs):
    """Host-side shard + layout prep. inputs: dict of full np arrays."""
    import ml_dtypes
    bf = ml_dtypes.bfloat16
    f8 = ml_dtypes.float8_e4m3
    f32 = np.float32
    sem = np.asarray(inputs["sem"], f32)
    pro = np.asarray(inputs["pro"], f32)

    def cols(v, nm):
        return np.ascontiguousarray(np.asarray(v, f32).reshape(nm, P).T)

    w1T = np.ascontiguousarray(np.asarray(inputs["W1"], f32).T)
    w2T = np.ascontiguousarray(np.asarray(inputs["W2"], f32).T)

    common = {
        "gsem": cols(inputs["g_sem"], MS),
        "gpro": cols(inputs["g_pro"], MP),
        "gff": cols(inputs["g_ff"], MS),
        "bq": cols(inputs["bq"], MS),
        "bk": cols(inputs["bk"], MS),
        "bo": cols(inputs["bo"], MS),
        "alpha": cols(inputs["alpha"], MF),
        "beta": cols(inputs["beta"], MF),
        "bvrow": np.ascontiguousarray(
            np.asarray(inputs["bv"], f32)[None, :]).astype(bf),
        "wqT": np.ascontiguousarray(np.asarray(inputs["Wq"], f32).T).astype(f8),
        "wkT": np.ascontiguousarray(np.asarray(inputs["Wk"], f32).T).astype(f8),
        "wvT": np.ascontiguousarray(np.asarray(inputs["Wv"], f32).T).astype(f8),
        "woT": np.ascontiguousarray(np.asarray(inputs["Wo"], f32).T).astype(f8),
    }

    in_maps = []
    for c in range(N_CORES):
        b, half = c // 2, c % 2
        m = dict(common)
        m["semT"] = np.ascontiguousarray(sem[b, half * TOK:(half + 1) * TOK, :].T)
        m["proT"] = np.ascontiguousarray(pro[b].T)
        m["w1shT"] = np.ascontiguousarray(w1T[:, c * DFS:(c + 1) * DFS])
        m["w2shT"] = np.ascontiguousarray(w2T[:, c * DSS:(c + 1) * DSS])
        in_maps.append(m)
    return in_maps


def assemble_out(results):
    out = np.empty((B, S, DS), np.float32)
    for c in range(N_CORES):
        b, half = c // 2, c % 2
        out[b, half * TOK:(half + 1) * TOK, :] = results[c]["outT"].T
    return out


def kernel(**inputs):
    nc = _get_nc()
    in_maps = make_in_maps(inputs)
    res = run_bass_kernel_spmd(nc, in_maps, core_ids=list(range(N_CORES)))
    return assemble_out(res.results)
